# revision 1
# baseline (speedup 1.0000x reference)
"""v2: on-device d[src] gather (ap_gather, 6 groups) + select-16 + multiply.

Per core: 2M edges in 82 calls x 6 groups x 4096 edges. Partition 16g+k of
group g holds d-slice k (d padded to 16*32768). ap_gather pulls d_k[lo] for
all k; a DVE is_equal mask against the per-partition slice id selects the
right slice, and a [96,6] block-ones matmul reduces the 16 candidates to the
true d[src], which is multiplied by matrix_values on-device.
Host: final np.add.at segment-sum + L1 (no device scatter primitive exists).
"""
import sys
sys.path.insert(0, "/opt/trn_rl_repo")
import numpy as np

N_NODES = 500_000
N_EDGES = 16_000_000
N_CORES = 8
E_CORE = N_EDGES // N_CORES          # 2_000_000
G = 6                                 # core groups used (0-5 verified exact)
PART = 16 * G                         # 96 partitions
NI = 3584                             # idxs per group per call
CALL_EDGES = G * NI                   # 24576
NCALLS = -(-E_CORE // CALL_EDGES)     # 82
E_PAD = NCALLS * CALL_EDGES           # 2_015_232
SLICE = 32768                         # d elems per partition slice
_RUNNER2 = None


def _build():
    import concourse.bass as bass
    import concourse.bacc as bacc
    import concourse.mybir as mybir
    from concourse import library_config

    nc = bacc.Bacc(None, target_bir_lowering=False)
    dtab = nc.dram_tensor("dtab", [PART, SLICE], mybir.dt.float32, kind="ExternalInput")
    gidx = nc.dram_tensor("gidx", [PART, NCALLS * (NI // 16)], mybir.dt.int16, kind="ExternalInput")
    hi = nc.dram_tensor("hi", [PART, NCALLS * NI], mybir.dt.float32, kind="ExternalInput")
    vals = nc.dram_tensor("vals", [G, NCALLS * NI], mybir.dt.float32, kind="ExternalInput")
    kconst = nc.dram_tensor("kconst", [PART, 1], mybir.dt.float32, kind="ExternalInput")
    onesblk = nc.dram_tensor("onesblk", [PART, G], mybir.dt.float32, kind="ExternalInput")
    contrib = nc.dram_tensor("contrib", [G, NCALLS * NI], mybir.dt.float32, kind="ExternalOutput")

    S16 = NI // 16
    with (
        nc.Block() as block,
        nc.semaphore("s_const") as s_const,
        nc.semaphore("s_in") as s_in,
        nc.semaphore("s_gath") as s_gath,
        nc.semaphore("s_mask") as s_mask,
        nc.semaphore("s_mm") as s_mm,
        nc.semaphore("s_ctb") as s_ctb,
        nc.semaphore("s_out") as s_out,
        nc.sbuf_tensor("dtab_sb", [PART, SLICE], mybir.dt.float32) as dtab_sb,
        nc.sbuf_tensor("kc_sb", [PART, 1], mybir.dt.float32) as kc_sb,
        nc.sbuf_tensor("ob_sb", [PART, G], mybir.dt.float32) as ob_sb,
        nc.sbuf_tensor("gi_sb", [PART, S16], mybir.dt.int16) as gi_sb,
        nc.sbuf_tensor("hi_sb", [PART, NI], mybir.dt.float32) as hi_sb,
        nc.sbuf_tensor("ga_sb", [PART, NI], mybir.dt.float32) as ga_sb,
        nc.sbuf_tensor("mk_sb", [PART, NI], mybir.dt.float32) as mk_sb,
        nc.sbuf_tensor("va_sb", [G, NI], mybir.dt.float32) as va_sb,
        nc.sbuf_tensor("ct_sb", [G, NI], mybir.dt.float32) as ct_sb,
        nc.psum_tensor("ps", [G, 512], mybir.dt.float32) as ps,
    ):
        NMM = NI // 512

        @block.sync
        def _(sync):
            sync.dma_start(dtab_sb[:, :], dtab.ap()).then_inc(s_const, 16)
            sync.dma_start(kc_sb[:, :], kconst.ap()).then_inc(s_const, 16)
            sync.dma_start(ob_sb[:, :], onesblk.ap()).then_inc(s_const, 16)
            for t in range(NCALLS):
                # serialize call t against consumption of buffers from t-1
                if t > 0:
                    sync.wait_ge(s_ctb, t * NMM)    # vals consumed
                    sync.wait_ge(s_gath, t)         # gidx consumed
                    sync.wait_ge(s_mask, t)         # hi consumed
                sync.dma_start(gi_sb[:, :], gidx.ap()[:, t * S16:(t + 1) * S16]).then_inc(s_in, 16)
                sync.dma_start(hi_sb[:, :], hi.ap()[:, t * NI:(t + 1) * NI]).then_inc(s_in, 16)
                sync.dma_start(va_sb[:, :], vals.ap()[:, t * NI:(t + 1) * NI]).then_inc(s_in, 16)
                sync.wait_ge(s_ctb, (t + 1) * NMM)
                sync.dma_start(contrib.ap()[:, t * NI:(t + 1) * NI], ct_sb[:, :]).then_inc(s_out, 16)
            sync.wait_ge(s_out, 16 * NCALLS)

        @block.gpsimd
        def _(g):
            g.load_library(library_config.ap_gather)
            g.wait_ge(s_const, 16)
            for t in range(NCALLS):
                g.wait_ge(s_in, 48 * t + 16)        # gidx of call t landed
                if t > 0:
                    g.wait_ge(s_mask, t)            # ga_sb consumed by masking of t-1
                g.ap_gather(
                    out_ap=ga_sb[:, :].rearrange("p (n d) -> p n d", d=1),
                    in_ap=dtab_sb[:, :].rearrange("p (n d) -> p n d", d=1),
                    idxs_ap=gi_sb[:, :],
                    channels=PART, num_elems=SLICE, d=1, num_idxs=NI,
                ).then_inc(s_gath, 1)

        @block.vector
        def _(vector):
            vector.wait_ge(s_const, 32)
            for t in range(NCALLS):
                vector.wait_ge(s_in, 48 * t + 32)   # hi landed
                vector.wait_ge(s_gath, t + 1)       # gather done
                vector.tensor_tensor(
                    out=mk_sb[:, :], in0=hi_sb[:, :],
                    in1=kc_sb[:, :1].to_broadcast([PART, NI]),
                    op=mybir.AluOpType.is_equal,
                )
                vector.tensor_tensor(
                    out=mk_sb[:, :], in0=mk_sb[:, :], in1=ga_sb[:, :],
                    op=mybir.AluOpType.mult,
                ).then_inc(s_mask, 1)
                # contrib = psum * vals, after matmuls per 512-chunk
                for m in range(NMM):
                    vector.wait_ge(s_mm, t * NMM + m + 1)
                    sl = slice(m * 512, (m + 1) * 512)
                    vector.wait_ge(s_in, 48 * t + 48)  # vals landed
                    vector.tensor_tensor(
                        out=ct_sb[:, sl], in0=ps[:, :], in1=va_sb[:, sl],
                        op=mybir.AluOpType.mult,
                    ).then_inc(s_ctb, 1)

        @block.tensor
        def _(tensor):
            tensor.wait_ge(s_const, 48)
            for t in range(NCALLS):
                tensor.wait_ge(s_mask, t + 1)
                for m in range(NMM):
                    if t > 0 or m > 0:
                        tensor.wait_ge(s_ctb, t * NMM + m)  # prev psum chunk consumed
                    sl = slice(m * 512, (m + 1) * 512)
                    tensor.matmul(
                        out=ps[:, :], lhsT=ob_sb[:, :], rhs=mk_sb[:, sl],
                        start=True, stop=True,
                    ).then_inc(s_mm, 1)

    nc.finalize()
    return nc


# ---- embedded SPMD runner ----
import time
import numpy as np
import jax
from jax.sharding import Mesh, PartitionSpec
from jax.experimental.shard_map import shard_map

import concourse.bass as bass
import concourse.mybir as mybir
from concourse import bass2jax
from concourse.bass2jax import _bass_exec_p, install_neuronx_cc_hook, partition_id_tensor


class SpmdRunner:
    def __init__(self, nc, n_cores=8):
        install_neuronx_cc_hook()
        self.nc = nc
        self.n_cores = n_cores
        assert nc.dbg_addr is None or not nc.dbg_callbacks
        partition_name = nc.partition_id_tensor.name if nc.partition_id_tensor else None
        in_names, out_names, out_avals, zero_outs = [], [], [], []
        for alloc in nc.m.functions[0].allocations:
            if not isinstance(alloc, mybir.MemoryLocationSet):
                continue
            name = alloc.memorylocations[0].name
            if alloc.kind == "ExternalInput":
                if name != partition_name and name != (nc.dbg_addr.name if nc.dbg_addr else None):
                    in_names.append(name)
            elif alloc.kind == "ExternalOutput":
                out_names.append(name)
                shape = tuple(alloc.tensor_shape)
                dtype = mybir.dt.np(alloc.dtype)
                out_avals.append(jax.core.ShapedArray(shape, dtype))
                zero_outs.append(np.zeros(shape, dtype))
        self.in_names, self.out_names = in_names, out_names
        self.out_avals, self.zero_outs = out_avals, zero_outs
        n_params, n_outs = len(in_names), len(out_avals)
        self.n_params = n_params

        all_in_names = list(in_names) + list(out_names)
        if nc.dbg_addr is not None:
            self.dbg_name = nc.dbg_addr.name
        else:
            self.dbg_name = None
        if partition_name is not None:
            all_in_names.append(partition_name)

        def _body(*args):
            operands = list(args)
            if partition_name is not None:
                operands.append(partition_id_tensor())
            outs = _bass_exec_p.bind(
                *operands,
                out_avals=tuple(out_avals),
                in_names=tuple(all_in_names),
                out_names=tuple(out_names),
                lowering_input_output_aliases=(),
                sim_require_finite=True,
                sim_require_nnan=True,
                nc=nc,
            )
            return tuple(outs)

        devices = jax.devices()[:n_cores]
        self.mesh = Mesh(np.asarray(devices), ("core",))
        in_specs = (PartitionSpec("core"),) * (n_params + n_outs)
        out_specs = (PartitionSpec("core"),) * n_outs
        # no donation so we can re-run with cached device inputs
        self.fn = jax.jit(
            shard_map(_body, mesh=self.mesh, in_specs=in_specs,
                      out_specs=out_specs, check_rep=False),
            keep_unused=True,
        )
        self._cached_dev_in = None

    def put_inputs(self, in_maps):
        """in_maps: list of n_cores dicts name->np array. Returns device arrays."""
        concat = [
            np.concatenate([np.asarray(in_maps[c][n]) for c in range(self.n_cores)], axis=0)
            for n in self.in_names
        ]
        concat += [
            np.zeros((self.n_cores * z.shape[0], *z.shape[1:]), z.dtype)
            for z in self.zero_outs
        ]
        self._cached_dev_in = jax.device_put(concat)
        return self._cached_dev_in

    def run(self, dev_in=None):
        dev_in = dev_in if dev_in is not None else self._cached_dev_in
        outs = self.fn(*dev_in)
        jax.block_until_ready(outs)
        return outs

    def results(self, outs):
        res = []
        for c in range(self.n_cores):
            m = {}
            for i, name in enumerate(self.out_names):
                a = np.asarray(outs[i]).reshape(self.n_cores, *self.out_avals[i].shape)
                m[name] = a[c]
            res.append(m)
        return res

    def time_runs(self, reps=5):
        ts = []
        for _ in range(reps):
            t0 = time.perf_counter()
            self.run()
            ts.append(time.perf_counter() - t0)
        return min(ts), ts


def _get_runner():
    global _RUNNER2
    if _RUNNER2 is None:
        _RUNNER2 = SpmdRunner(_build(), N_CORES)
    return _RUNNER2

_get_runner2 = _get_runner


def _prep_core(src, dstv, valv, d_pad):
    """Returns in_map plus blocked dst array for host combine."""
    ns = E_PAD - len(src)
    srcp = np.concatenate([src, np.zeros(ns, np.int32)])
    dstp = np.concatenate([dstv, np.zeros(ns, np.int32)])
    valp = np.concatenate([valv, np.zeros(ns, np.float32)])
    # block layout: call t, group g, edge j -> flat (t*G+g)*NI + j
    lo = (srcp & (SLICE - 1)).astype(np.int16)
    hi = (srcp >> 15).astype(np.float32)
    lo_b = lo.reshape(NCALLS, G, NI)
    hi_b = hi.reshape(NCALLS, G, NI)
    val_b = valp.reshape(NCALLS, G, NI)
    # gidx [PART, NCALLS*NI/16]: group g partitions 16g+p hold wrapped lo
    gidx = np.zeros((PART, NCALLS * (NI // 16)), np.int16)
    hiA = np.zeros((PART, NCALLS * NI), np.float32)
    vals = np.zeros((G, NCALLS * NI), np.float32)
    for t in range(NCALLS):
        for g in range(G):
            w = lo_b[t, g].reshape(NI // 16, 16).T       # [16, NI/16]
            gidx[16 * g:16 * g + 16, t * (NI // 16):(t + 1) * (NI // 16)] = w
            hiA[16 * g:16 * g + 16, t * NI:(t + 1) * NI] = hi_b[t, g][None, :]
            vals[g, t * NI:(t + 1) * NI] = val_b[t, g]
    kconst = (np.arange(PART) % 16).astype(np.float32).reshape(PART, 1)
    onesblk = np.zeros((PART, G), np.float32)
    for p in range(PART):
        onesblk[p, p // 16] = 1.0
    return {
        "dtab": np.tile(d_pad.reshape(16, SLICE), (G, 1)),
        "gidx": gidx, "hi": hiA, "vals": vals,
        "kconst": kconst, "onesblk": onesblk,
    }, dstp.reshape(NCALLS, G, NI)


def kernel(d, edge_index, matrix_values, mask, residual):
    d = np.asarray(d, dtype=np.float32)
    edge_index = np.asarray(edge_index)
    matrix_values = np.asarray(matrix_values, dtype=np.float32)
    mask = np.asarray(mask)
    residual = np.asarray(residual, dtype=np.float32)
    dst = edge_index[0].astype(np.int32)
    src = edge_index[1].astype(np.int32)
    d_pad = np.concatenate([d, np.zeros(16 * SLICE - N_NODES, np.float32)])

    in_maps, dst_blocks = [], []
    for c in range(N_CORES):
        sl = slice(c * E_CORE, (c + 1) * E_CORE)
        m, dstb = _prep_core(src[sl], dst[sl], matrix_values[sl], d_pad)
        in_maps.append(m)
        dst_blocks.append(dstb)

    r = _get_runner2()
    r.put_inputs(in_maps)
    outs = r.run()
    res = r.results(outs)

    Ad = np.zeros(N_NODES, np.float32)
    for c in range(N_CORES):
        ctb = res[c]["contrib"].reshape(G, NCALLS, NI).transpose(1, 0, 2)  # [t, g, j]
        np.add.at(Ad, dst_blocks[c].ravel(), ctb.ravel())
    Ad = np.where(mask, Ad, np.float32(0))
    return np.asarray(np.mean(np.abs(Ad - residual)), dtype=np.float32)



# revision 2
# speedup vs baseline: 2.2669x; 2.2669x over previous
"""v3: src-resharded node-slot gather.

Edges are resharded by src range: core c owns nodes [c*62528, (c+1)*62528),
so every edge of a node lands on one core (global degree ~Poisson(32)).
Within a core, q7 core g of the gpsimd engine holds the f32 d-subslice of
7816 nodes on each of its 16 partitions. Edges of a node are packed into
"slot columns" of up to 16 (one per partition row); ap_gather broadcasts
d[node] to all 16 rows of a column in one index, so the 16-way partition
redundancy of ap_gather performs the edge expansion for free. A DVE
multiply by the host-laid-out matrix_values gives per-edge contributions,
which stream back to HBM; the host finishes with a bincount segment-sum,
mask, and L1 mean (as in the accepted baseline).
"""
import sys
sys.path.insert(0, "/opt/trn_rl_repo")
import numpy as np

N_NODES = 500_000
N_CORES = 8
NODES_CORE = 62528            # per-core node range (8 * 7816)
NODES_Q7 = 7816               # nodes per q7 core (gather table size)
N_PAD = NODES_CORE * N_CORES  # 500224 padded node count
ROWS = 16                     # edge slots per column (partition rows per q7 core)
T_CHUNKS = 4                  # ap_gather calls (chunked for DVE overlap)

_RUNNER2 = None
_S = None


def _build(S):
    import concourse.bass as bass
    import concourse.bacc as bacc
    import concourse.mybir as mybir
    from concourse import library_config

    NI = S // T_CHUNKS
    S16 = NI // 16
    nc = bacc.Bacc(None, target_bir_lowering=False)
    dtab = nc.dram_tensor("dtab", [128, NODES_Q7], mybir.dt.float32, kind="ExternalInput")
    gidx = nc.dram_tensor("gidx", [128, S // 16], mybir.dt.int16, kind="ExternalInput")
    vals = nc.dram_tensor("vals", [128, S], mybir.dt.float32, kind="ExternalInput")
    contrib = nc.dram_tensor("contrib", [128, S], mybir.dt.float32, kind="ExternalOutput")

    with (
        nc.Block() as block,
        nc.semaphore("s_in") as s_in,
        nc.semaphore("s_g") as s_g,
        nc.semaphore("s_mul") as s_mul,
        nc.semaphore("s_out") as s_out,
        nc.sbuf_tensor("tab_sb", [128, NODES_Q7], mybir.dt.float32) as tab_sb,
        nc.sbuf_tensor("gi_sb", [128, S // 16], mybir.dt.int16) as gi_sb,
        nc.sbuf_tensor("va_sb", [128, S], mybir.dt.float32) as va_sb,
        nc.sbuf_tensor("ga_sb", [128, S], mybir.dt.float32) as ga_sb,
    ):
        @block.sync
        def _(sync):
            sync.dma_start(tab_sb[:, :], dtab.ap()).then_inc(s_in, 16)
            sync.dma_start(gi_sb[:, :], gidx.ap()).then_inc(s_in, 16)
            sync.dma_start(va_sb[:, :], vals.ap()).then_inc(s_in, 16)
            for t in range(T_CHUNKS):
                sync.wait_ge(s_mul, t + 1)
                sync.dma_start(
                    contrib.ap()[:, t * NI:(t + 1) * NI], va_sb[:, t * NI:(t + 1) * NI]
                ).then_inc(s_out, 16)
            sync.wait_ge(s_out, 16 * T_CHUNKS)

        @block.gpsimd
        def _(g):
            g.load_library(library_config.ap_gather)
            g.wait_ge(s_in, 48)
            for t in range(T_CHUNKS):
                g.ap_gather(
                    out_ap=ga_sb[:, t * NI:(t + 1) * NI].rearrange("p (n d) -> p n d", d=1),
                    in_ap=tab_sb[:, :].rearrange("p (n d) -> p n d", d=1),
                    idxs_ap=gi_sb[:, t * S16:(t + 1) * S16],
                    channels=128, num_elems=NODES_Q7, d=1, num_idxs=NI,
                ).then_inc(s_g, 1)

        @block.vector
        def _(vector):
            vector.wait_ge(s_in, 48)
            for t in range(T_CHUNKS):
                vector.wait_ge(s_g, t + 1)
                sl = slice(t * NI, (t + 1) * NI)
                vector.tensor_tensor(
                    out=va_sb[:, sl], in0=ga_sb[:, sl], in1=va_sb[:, sl],
                    op=mybir.AluOpType.mult,
                ).then_inc(s_mul, 1)

    nc.finalize()
    return nc


# ---- embedded SPMD runner ----
import time
import jax
from jax.sharding import Mesh, PartitionSpec
from jax.experimental.shard_map import shard_map

import concourse.bass as bass
import concourse.mybir as mybir
from concourse import bass2jax
from concourse.bass2jax import _bass_exec_p, install_neuronx_cc_hook, partition_id_tensor


class SpmdRunner:
    def __init__(self, nc, n_cores=8):
        install_neuronx_cc_hook()
        self.nc = nc
        self.n_cores = n_cores
        assert nc.dbg_addr is None or not nc.dbg_callbacks
        partition_name = nc.partition_id_tensor.name if nc.partition_id_tensor else None
        in_names, out_names, out_avals, zero_outs = [], [], [], []
        for alloc in nc.m.functions[0].allocations:
            if not isinstance(alloc, mybir.MemoryLocationSet):
                continue
            name = alloc.memorylocations[0].name
            if alloc.kind == "ExternalInput":
                if name != partition_name and name != (nc.dbg_addr.name if nc.dbg_addr else None):
                    in_names.append(name)
            elif alloc.kind == "ExternalOutput":
                out_names.append(name)
                shape = tuple(alloc.tensor_shape)
                dtype = mybir.dt.np(alloc.dtype)
                out_avals.append(jax.core.ShapedArray(shape, dtype))
                zero_outs.append(np.zeros(shape, dtype))
        self.in_names, self.out_names = in_names, out_names
        self.out_avals, self.zero_outs = out_avals, zero_outs
        n_params, n_outs = len(in_names), len(out_avals)
        self.n_params = n_params

        all_in_names = list(in_names) + list(out_names)
        if nc.dbg_addr is not None:
            self.dbg_name = nc.dbg_addr.name
        else:
            self.dbg_name = None
        if partition_name is not None:
            all_in_names.append(partition_name)

        def _body(*args):
            operands = list(args)
            if partition_name is not None:
                operands.append(partition_id_tensor())
            outs = _bass_exec_p.bind(
                *operands,
                out_avals=tuple(out_avals),
                in_names=tuple(all_in_names),
                out_names=tuple(out_names),
                lowering_input_output_aliases=(),
                sim_require_finite=True,
                sim_require_nnan=True,
                nc=nc,
            )
            return tuple(outs)

        devices = jax.devices()[:n_cores]
        self.mesh = Mesh(np.asarray(devices), ("core",))
        in_specs = (PartitionSpec("core"),) * (n_params + n_outs)
        out_specs = (PartitionSpec("core"),) * n_outs
        # no donation so we can re-run with cached device inputs
        self.fn = jax.jit(
            shard_map(_body, mesh=self.mesh, in_specs=in_specs,
                      out_specs=out_specs, check_rep=False),
            keep_unused=True,
        )
        self._cached_dev_in = None

    def put_inputs(self, in_maps):
        """in_maps: list of n_cores dicts name->np array. Returns device arrays."""
        concat = [
            np.concatenate([np.asarray(in_maps[c][n]) for c in range(self.n_cores)], axis=0)
            for n in self.in_names
        ]
        concat += [
            np.zeros((self.n_cores * z.shape[0], *z.shape[1:]), z.dtype)
            for z in self.zero_outs
        ]
        self._cached_dev_in = jax.device_put(concat)
        return self._cached_dev_in

    def run(self, dev_in=None):
        dev_in = dev_in if dev_in is not None else self._cached_dev_in
        outs = self.fn(*dev_in)
        jax.block_until_ready(outs)
        return outs

    def results(self, outs):
        res = []
        for c in range(self.n_cores):
            m = {}
            for i, name in enumerate(self.out_names):
                a = np.asarray(outs[i]).reshape(self.n_cores, *self.out_avals[i].shape)
                m[name] = a[c]
            res.append(m)
        return res

    def time_runs(self, reps=5):
        ts = []
        for _ in range(reps):
            t0 = time.perf_counter()
            self.run()
            ts.append(time.perf_counter() - t0)
        return min(ts), ts


def _get_runner(S=None):
    global _RUNNER2, _S
    if _RUNNER2 is None:
        assert S is not None, "call kernel() first"
        _RUNNER2 = SpmdRunner(_build(S), N_CORES)
        _S = S
    return _RUNNER2

_get_runner2 = _get_runner


def _prep_core(s_local, v, dstv, S):
    """Build per-core layouts. s_local: local src ids sorted ascending."""
    E_c = len(s_local)
    deg = np.bincount(s_local, minlength=NODES_CORE)
    slots = (deg + ROWS - 1) // ROWS
    cs_deg = np.concatenate([[0], np.cumsum(deg)[:-1]])      # exclusive cumsum
    cs_slot = np.concatenate([[0], np.cumsum(slots)[:-1]])
    node_g = np.arange(NODES_CORE) // NODES_Q7               # q7 core of node
    g_slot_base = cs_slot[node_g * NODES_Q7]                 # slots before this g
    colstart = cs_slot - g_slot_base                         # column of node within g

    r = np.arange(E_c) - cs_deg[s_local]                     # rank of edge within node
    g = node_g[s_local]
    col = colstart[s_local] + r // ROWS
    row = r % ROWS

    vals_layout = np.zeros((128, S), np.float32)
    dst_layout = np.full((128, S), N_PAD, np.int32)
    pidx = (16 * g + row).astype(np.int64)
    flat = pidx * S + col
    vals_layout.reshape(-1)[flat] = v
    dst_layout.reshape(-1)[flat] = dstv

    # gather index list per q7 core: column j of g -> local node idx, wrapped
    # per chunk: idx i of chunk t stored at [16g + i%16, t*(NI/16) + i//16]
    NI = S // T_CHUNKS
    gidx = np.zeros((128, S // 16), np.int16)
    for gq in range(8):
        nodes_g = np.arange(gq * NODES_Q7, (gq + 1) * NODES_Q7)
        I_g = np.repeat(nodes_g - gq * NODES_Q7, slots[nodes_g]).astype(np.int16)
        I_g = np.concatenate([I_g, np.zeros(S - len(I_g), np.int16)])
        w = I_g.reshape(T_CHUNKS, NI // 16, 16).transpose(0, 2, 1)  # [T, 16, NI/16]
        gidx[16 * gq:16 * gq + 16, :] = w.transpose(1, 0, 2).reshape(16, S // 16)
    return vals_layout, dst_layout, gidx


def kernel(d, edge_index, matrix_values, mask, residual):
    d = np.asarray(d, dtype=np.float32)
    edge_index = np.asarray(edge_index)
    matrix_values = np.asarray(matrix_values, dtype=np.float32)
    mask = np.asarray(mask)
    residual = np.asarray(residual, dtype=np.float32)
    dst = edge_index[0].astype(np.int32)
    src = edge_index[1].astype(np.int32)
    d_pad = np.concatenate([d, np.zeros(N_PAD - N_NODES, np.float32)])

    # reshard edges by src range; sort by src groups cores and nodes at once
    order = np.argsort(src, kind="stable")
    src_s = src[order]
    dst_s = dst[order]
    val_s = matrix_values[order]
    bounds = np.searchsorted(src_s, np.arange(N_CORES + 1) * NODES_CORE)

    # S = max slot count over (core, q7): slots of all nodes from global degree
    deg_all = np.bincount(src_s, minlength=N_PAD)
    slots_all = (deg_all + ROWS - 1) // ROWS
    S_need = int(slots_all.reshape(N_CORES, 8, NODES_Q7).sum(axis=2).max())
    S = -(-S_need // (16 * T_CHUNKS)) * (16 * T_CHUNKS)

    in_maps, dst_layouts = [], []
    for c in range(N_CORES):
        e0, e1 = bounds[c], bounds[c + 1]
        s_local = (src_s[e0:e1] - c * NODES_CORE).astype(np.int64)
        vals_layout, dst_layout, gidx = _prep_core(
            s_local, val_s[e0:e1], dst_s[e0:e1], S)
        dtab = np.tile(
            np.repeat(d_pad[c * NODES_CORE:(c + 1) * NODES_CORE].reshape(8, NODES_Q7),
                      16, axis=0), (1, 1))
        in_maps.append({"dtab": dtab, "gidx": gidx, "vals": vals_layout})
        dst_layouts.append(dst_layout)

    r = _get_runner(S)
    r.put_inputs(in_maps)
    outs = r.run()
    res = r.results(outs)

    Ad = np.zeros(N_PAD + 1, np.float64)
    for c in range(N_CORES):
        ctb = res[c]["contrib"]
        Ad += np.bincount(dst_layouts[c].ravel(), weights=ctb.ravel().astype(np.float64),
                          minlength=N_PAD + 1)
    Ad = Ad[:N_NODES].astype(np.float32)
    Ad = np.where(mask, Ad, np.float32(0))
    return np.asarray(np.mean(np.abs(Ad - residual)), dtype=np.float32)
